# revision 1
# baseline (speedup 1.0000x reference)
"""ConstMultiHeadGAT forward on 8 TRN2 NeuronCores.

Math: attention logits are all zero, so softmax gives uniform 1/deg(row)
weights.  For each head i:
    out[:, i*64:(i+1)*64] = diag(1/deg) @ A @ h @ W[i]
where A is the edge-incidence matrix (row <- col).  Aggregation commutes
with the projection, so we aggregate raw 256-wide features first (halving
gather traffic vs. gathering 512-wide projected features) and project once
per node:
    agg[n, :] = sum_{e: row_e = n} h[col_e, :]
    out[n, :] = (1/deg[n]) * agg[n, :] @ Wcat          (Wcat = [256, 512])

Distribution: nodes are split into 8 contiguous shards of 12500; each edge
is routed to the core that owns its destination (row).  No collectives —
each core independently gathers h[col] (h replicated in each core's HBM,
stored fp16 to halve gather traffic), segment-sums into its node shard in
fp32 PSUM, projects in fp32, and writes its fp32 output shard.

Per core the node shard is processed in 49 superblocks of 2 tiles x 128
nodes.  Edges are grouped by (superblock, source bank, tile); dma_gather
indices are signed int16 so h is addressed as 4 banks of 25000 rows; each
(tile, bank) group is padded to whole chunks of 128 edges.  Per superblock:
  - one dma_gather per non-empty bank pulls both tiles' edge rows:
      feat[i%128, i//128, :] = h_bank[idx[i], :]        (fp16)
  - one broadcast is_equal builds all chunk one-hot matrices
      sel[p, j, n] = (row_local[p, j] == n)             (fp16, exact 0/1)
  - per chunk two fp16 matmuls segment-sum into the owning tile's fp32
    PSUM accumulators: aggT[d, n] += feat_chunk^T @ sel_chunk
  - per tile: project aggT through Wcat (fp32), scale by 1/deg, DMA out.
Chunk counts per (superblock, bank, tile) are maxed over the 8 cores so a
single SPMD program serves all cores; pad slots gather bank row 0 and have
row_local = 32768 so they contribute nothing.
"""

import math

import numpy as np

import concourse.bacc as bacc
import concourse.bass as bass
import concourse.mybir as mybir
import concourse.tile as tile
from concourse.bass_utils import run_bass_kernel_spmd

N_NODES = 100000
N_EDGES = 1600000
D_IN = 256
D_OUT = 64
N_HEADS = 8
N_CORES = 8
NODES_PER_CORE = N_NODES // N_CORES  # 12500
P = 128
T_TILES = math.ceil(NODES_PER_CORE / P)  # 98
SB_TILES = 2  # tiles per superblock (gather batching)
N_SB = math.ceil(T_TILES / SB_TILES)  # 49
DCAT = N_HEADS * D_OUT  # 512
N_BANKS = 4
BANK = 25000  # dma_gather idx is signed int16 -> banks < 32768 rows
F32 = mybir.dt.float32
F16 = mybir.dt.float16
I16 = mybir.dt.int16
NP_FEAT = np.float16


def _build_program(n_table, bank_rows, K, off, num_devices, repeat=1):
    """K[g], off[g] for group g = (sb*N_BANKS + b)*SB_TILES + tpos.
    One SPMD program for all cores.  repeat>1 wraps the body in a device
    loop (used only for timing measurements)."""
    nc = bacc.Bacc(
        "TRN2", target_bir_lowering=False, debug=False, num_devices=num_devices
    )
    chunks_total = int(sum(K))
    n_banks = N_BANKS

    # h holds fp16 payload declared as f32[*, D_IN//2]: the gather ucode
    # crashes on 2-byte dtypes, so we move 512B rows as f32 and bitcast the
    # gathered SBUF bytes back to fp16 for compute.
    h_d = nc.dram_tensor("h", [n_table, D_IN // 2], F32, kind="ExternalInput")
    cols_d = nc.dram_tensor("cols", [P, chunks_total * 8], I16, kind="ExternalInput")
    rows_d = nc.dram_tensor("rows", [P, chunks_total], F16, kind="ExternalInput")
    invdeg_d = nc.dram_tensor("invdeg", [P, T_TILES], F32, kind="ExternalInput")
    wcat_d = nc.dram_tensor("wcat", [2, P, DCAT], F32, kind="ExternalInput")
    iota_d = nc.dram_tensor("iota", [P, P], F16, kind="ExternalInput")
    out_d = nc.dram_tensor("out", [T_TILES * P, DCAT], F32, kind="ExternalOutput")

    def body(tc, cpool, fpool, spool, apool, opool, ppool, consts):
        cols_sb, rows_sb, invdeg_sb, w0_sb, w1_sb, iota_sb = consts
        for sb in range(N_SB):
            tiles = [t for t in range(sb * SB_TILES, min((sb + 1) * SB_TILES, T_TILES))]
            g0 = (sb * n_banks) * SB_TILES
            g_end = ((sb + 1) * n_banks) * SB_TILES
            base = off[g0]
            k_sb = int(sum(K[g0:g_end]))
            if k_sb == 0:
                continue
            feat = fpool.tile([P, k_sb * (D_IN // 2)], F32, tag="feat")
            featv = feat[:].bitcast(F16)  # [P, k_sb * D_IN] fp16 view
            for b in range(n_banks):
                gb = (sb * n_banks + b) * SB_TILES
                kb = int(sum(K[gb : gb + SB_TILES]))
                if kb == 0:
                    continue
                rel = off[gb] - base
                dh = D_IN // 2
                nc.gpsimd.dma_gather(
                    out_ap=feat[:, rel * dh : (rel + kb) * dh].rearrange(
                        "p (c d) -> p c d", c=kb
                    ),
                    in_ap=h_d.ap()[
                        b * bank_rows : min((b + 1) * bank_rows, n_table), :
                    ],
                    idxs_ap=cols_sb[:, off[gb] * 8 : (off[gb] + kb) * 8],
                    num_idxs=kb * P,
                    num_idxs_reg=kb * P,
                    elem_size=dh,
                    single_packet=False,
                )
            sel = spool.tile([P, k_sb * P], F16, tag="sel")
            nc.vector.tensor_tensor(
                out=sel[:].rearrange("p (k n) -> p k n", k=k_sb),
                in0=iota_sb[:].unsqueeze(1).to_broadcast([P, k_sb, P]),
                in1=rows_sb[:, base : base + k_sb]
                .unsqueeze(2)
                .to_broadcast([P, k_sb, P]),
                op=mybir.AluOpType.is_equal,
            )
            # chunk -> tile-position map and first/last chunk per tpos
            chunk_tpos = []
            for b in range(n_banks):
                for tpos in range(len(tiles)):
                    g = (sb * n_banks + b) * SB_TILES + tpos
                    chunk_tpos += [tpos] * int(K[g])
            first = {}
            last = {}
            for j, tp in enumerate(chunk_tpos):
                first.setdefault(tp, j)
                last[tp] = j
            psums = {}
            for tp in sorted(first):
                p0 = ppool.tile([P, P], F32, tag="p0")
                p1 = ppool.tile([P, P], F32, tag="p1")
                psums[tp] = (p0, p1)
            for j, tp in enumerate(chunk_tpos):
                fj = featv[:, j * D_IN : (j + 1) * D_IN]
                sj = sel[:, j * P : (j + 1) * P]
                p0, p1 = psums[tp]
                nc.tensor.matmul(
                    out=p0[:],
                    lhsT=fj[:, :P],
                    rhs=sj,
                    start=(j == first[tp]),
                    stop=(j == last[tp]),
                )
                nc.tensor.matmul(
                    out=p1[:],
                    lhsT=fj[:, P:],
                    rhs=sj,
                    start=(j == first[tp]),
                    stop=(j == last[tp]),
                )
            for tp in sorted(first):
                t = tiles[tp]
                p0, p1 = psums[tp]
                a0 = apool.tile([P, P], F32, tag="a0")
                a1 = apool.tile([P, P], F32, tag="a1")
                nc.vector.tensor_copy(out=a0[:], in_=p0[:])
                nc.vector.tensor_copy(out=a1[:], in_=p1[:])
                po = ppool.tile([P, DCAT], F32, tag="po")
                nc.tensor.matmul(
                    out=po[:], lhsT=a0[:], rhs=w0_sb[:], start=True, stop=False
                )
                nc.tensor.matmul(
                    out=po[:], lhsT=a1[:], rhs=w1_sb[:], start=False, stop=True
                )
                ot = opool.tile([P, DCAT], F32, tag="ot")
                nc.vector.tensor_scalar(
                    out=ot[:],
                    in0=po[:],
                    scalar1=invdeg_sb[:, t : t + 1],
                    scalar2=None,
                    op0=mybir.AluOpType.mult,
                )
                nc.sync.dma_start(out=out_d.ap()[t * P : (t + 1) * P, :], in_=ot[:])

    with tile.TileContext(nc) as tc:
        with (
            tc.tile_pool(name="const", bufs=1) as cpool,
            tc.tile_pool(name="feat", bufs=3) as fpool,
            tc.tile_pool(name="sel", bufs=3) as spool,
            tc.tile_pool(name="agg", bufs=2) as apool,
            tc.tile_pool(name="outp", bufs=3) as opool,
            tc.tile_pool(name="psum", bufs=2, space="PSUM") as ppool,
        ):
            cols_sb = cpool.tile([P, chunks_total * 8], I16)
            rows_sb = cpool.tile([P, chunks_total], F16)
            invdeg_sb = cpool.tile([P, T_TILES], F32)
            w0_sb = cpool.tile([P, DCAT], F32, tag="w0")
            w1_sb = cpool.tile([P, DCAT], F32, tag="w1")
            iota_sb = cpool.tile([P, P], F16)
            nc.sync.dma_start(out=cols_sb[:], in_=cols_d.ap())
            nc.sync.dma_start(out=rows_sb[:], in_=rows_d.ap())
            nc.sync.dma_start(out=invdeg_sb[:], in_=invdeg_d.ap())
            nc.sync.dma_start(out=w0_sb[:], in_=wcat_d.ap()[0])
            nc.sync.dma_start(out=w1_sb[:], in_=wcat_d.ap()[1])
            nc.sync.dma_start(out=iota_sb[:], in_=iota_d.ap())
            consts = (cols_sb, rows_sb, invdeg_sb, w0_sb, w1_sb, iota_sb)
            if repeat == 1:
                body(tc, cpool, fpool, spool, apool, opool, ppool, consts)
            else:
                with tc.For_i(0, repeat, 1):
                    body(tc, cpool, fpool, spool, apool, opool, ppool, consts)
    nc.compile()
    return nc


def _prep_inputs(h, W, edge_idx, n_nodes, nodes_per_core, n_cores, bank_rows):
    h16 = np.ascontiguousarray(np.asarray(h, dtype=np.float32).astype(NP_FEAT))
    h_wire = h16.view(np.float32)  # fp16 payload shipped as f32[N, D_IN//2]
    W = np.asarray(W, dtype=np.float32)
    ei = np.asarray(edge_idx)
    row = ei[0].astype(np.int64)
    col = ei[1].astype(np.int64)
    e = row.shape[0]
    n_banks = N_BANKS

    deg = np.bincount(row, minlength=n_nodes).astype(np.float64)
    invdeg_full = np.where(deg > 0, 1.0 / np.maximum(deg, 1.0), 0.0).astype(np.float32)

    core = row // nodes_per_core
    local = row - core * nodes_per_core
    tloc = local // P
    slot = local - tloc * P
    sbk = tloc // SB_TILES
    tpos = tloc - sbk * SB_TILES
    bank = col // bank_rows
    colloc = (col - bank * bank_rows).astype(np.int16)

    n_groups = N_SB * n_banks * SB_TILES
    gkey_local = (sbk * n_banks + bank) * SB_TILES + tpos
    gkey = core * n_groups + gkey_local
    order = np.argsort(gkey, kind="stable")
    gkey_s = gkey[order]
    counts = np.bincount(gkey_s, minlength=n_cores * n_groups).reshape(
        n_cores, n_groups
    )
    K = np.ceil(counts.max(axis=0) / P).astype(np.int64)  # [n_groups]
    off = np.zeros(n_groups, dtype=np.int64)
    np.cumsum(K[:-1], out=off[1:])
    chunks_total = int(K.sum())

    starts = np.zeros(n_cores * n_groups + 1, dtype=np.int64)
    np.cumsum(counts.reshape(-1), out=starts[1:])
    pos = np.arange(e, dtype=np.int64) - starts[gkey_s]
    chunkpos = pos // P
    part = pos - chunkpos * P

    g_s = gkey_local[order]
    gchunk = off[g_s] + chunkpos

    # gather-call grouping: call (sb, b) covers groups [gb, gb+SB_TILES);
    # idx position within the call is relative to off[gb].
    gb_s = (g_s // SB_TILES) * SB_TILES
    i_call = (gchunk - off[gb_s]) * P + part

    cols_arr = np.zeros((n_cores, P, chunks_total * 8), dtype=np.int16)
    rows_arr = np.full((n_cores, P, chunks_total), 32768.0, dtype=NP_FEAT)
    core_s = core[order]
    idx_col = off[gb_s] * 8 + i_call // 16
    idx_row = i_call % 16
    colloc_s = colloc[order]
    for grp in range(8):  # replicate across the 8 q7 cores
        cols_arr[core_s, grp * 16 + idx_row, idx_col] = colloc_s
    rows_arr[core_s, part, gchunk] = slot[order].astype(NP_FEAT)

    invdeg_arr = np.zeros((n_cores, P, T_TILES), dtype=np.float32)
    nloc = np.arange(nodes_per_core, dtype=np.int64)
    tt = nloc // P
    ss = nloc - tt * P
    for c in range(n_cores):
        invdeg_arr[c, ss, tt] = invdeg_full[c * nodes_per_core + nloc]

    wcat = np.ascontiguousarray(
        W.transpose(1, 0, 2).reshape(D_IN, DCAT).reshape(2, P, DCAT)
    )
    iota_row = np.tile(np.arange(P, dtype=NP_FEAT)[None, :], (P, 1))

    in_maps = [
        {
            "h": h_wire,
            "cols": cols_arr[c],
            "rows": rows_arr[c],
            "invdeg": invdeg_arr[c],
            "wcat": wcat,
            "iota": iota_row,
        }
        for c in range(n_cores)
    ]
    return in_maps, K.tolist(), off.tolist()


_NC_CACHE: dict = {}


def kernel(h, W, edge_idx) -> np.ndarray:
    in_maps, K, off = _prep_inputs(
        h, W, edge_idx, N_NODES, NODES_PER_CORE, N_CORES, BANK
    )
    key = tuple(K)
    if key not in _NC_CACHE:
        _NC_CACHE[key] = _build_program(N_NODES, BANK, K, off, N_CORES)
    nc = _NC_CACHE[key]
    res = run_bass_kernel_spmd(nc, in_maps, core_ids=list(range(N_CORES)))
    out = np.concatenate(
        [res.results[c]["out"][:NODES_PER_CORE] for c in range(N_CORES)], axis=0
    )
    return np.ascontiguousarray(out.astype(np.float32))



# revision 4
# speedup vs baseline: 3.1145x; 3.1145x over previous
"""ConstMultiHeadGAT forward on 8 TRN2 NeuronCores.

Math: attention logits are all zero, so softmax gives uniform 1/deg(row)
weights.  For each head i:
    out[:, i*64:(i+1)*64] = diag(1/deg) @ A @ h @ W[i]
where A is the edge-incidence matrix (row <- col).  Aggregation commutes
with the projection, so we aggregate raw 256-wide features first (halving
gather traffic vs. gathering 512-wide projected features) and project once
per node:
    agg[n, :] = sum_{e: row_e = n} h[col_e, :]
    out[n, :] = (1/deg[n]) * agg[n, :] @ Wcat          (Wcat = [256, 512])

Distribution: nodes are split into 8 contiguous shards of 12500; each edge
is routed to the core that owns its destination (row).  No collectives —
each core independently gathers h[col] (h replicated in each core's HBM,
stored fp16 to halve gather traffic), segment-sums into its node shard in
fp32 PSUM, projects in fp32, and writes its fp32 output shard.

Per core the node shard is processed in 49 superblocks of 2 tiles x 128
nodes.  Edges are grouped by (superblock, source bank, tile); dma_gather
indices are signed int16 so h is addressed as 4 banks of 25000 rows; each
(tile, bank) group is padded to whole chunks of 128 edges.  Per superblock:
  - one dma_gather per non-empty bank pulls both tiles' edge rows:
      feat[i%128, i//128, :] = h_bank[idx[i], :]        (fp16)
  - one broadcast is_equal builds all chunk one-hot matrices
      sel[p, j, n] = (row_local[p, j] == n)             (fp16, exact 0/1)
  - per chunk two fp16 matmuls segment-sum into the owning tile's fp32
    PSUM accumulators: aggT[d, n] += feat_chunk^T @ sel_chunk
  - per tile: project aggT through Wcat (fp32), scale by 1/deg, DMA out.
Chunk counts per (superblock, bank, tile) are maxed over the 8 cores so a
single SPMD program serves all cores; pad slots gather bank row 0 and have
row_local = 32768 so they contribute nothing.
"""

import math

import numpy as np

import concourse.bacc as bacc
import concourse.bass as bass
import concourse.mybir as mybir
import concourse.tile as tile
from concourse.bass_utils import run_bass_kernel_spmd

N_NODES = 100000
N_EDGES = 1600000
D_IN = 256
D_OUT = 64
N_HEADS = 8
N_CORES = 8
NODES_PER_CORE = N_NODES // N_CORES  # 12500
P = 128
T_TILES = math.ceil(NODES_PER_CORE / P)  # 98
SB_TILES = 2  # tiles per superblock (gather batching)
N_SB = math.ceil(T_TILES / SB_TILES)  # 49
DCAT = N_HEADS * D_OUT  # 512
N_BANKS = 4
BANK = 25000  # dma_gather idx is signed int16 -> banks < 32768 rows
F32 = mybir.dt.float32
F16 = mybir.dt.float16
I16 = mybir.dt.int16
NP_FEAT = np.float16


def _build_program(n_table, bank_rows, K, off, num_devices, repeat=1):
    """K[g], off[g] for group g = (sb*N_BANKS + b)*SB_TILES + tpos.
    One SPMD program for all cores.  repeat>1 wraps the body in a device
    loop (used only for timing measurements)."""
    nc = bacc.Bacc(
        "TRN2",
        target_bir_lowering=False,
        debug=False,
        num_devices=num_devices,
        num_swdge_queues=4,
    )
    chunks_total = int(sum(K))
    n_banks = N_BANKS

    # h holds fp16 payload declared as f32[*, D_IN//2]: the gather ucode
    # crashes on 2-byte dtypes, so we move 512B rows as f32 and bitcast the
    # gathered SBUF bytes back to fp16 for compute.
    h_d = nc.dram_tensor("h", [n_table, D_IN // 2], F32, kind="ExternalInput")
    cols_d = nc.dram_tensor("cols", [P, chunks_total * 8], I16, kind="ExternalInput")
    rows_d = nc.dram_tensor("rows", [P, chunks_total], F16, kind="ExternalInput")
    invdeg_d = nc.dram_tensor("invdeg", [P, T_TILES], F32, kind="ExternalInput")
    wcat_d = nc.dram_tensor("wcat", [2, P, DCAT], F32, kind="ExternalInput")
    iota_d = nc.dram_tensor("iota", [P, P], F16, kind="ExternalInput")
    out_d = nc.dram_tensor("out", [T_TILES * P, DCAT], F32, kind="ExternalOutput")

    def body(tc, cpool, fpool, spool, apool, opool, ppool, consts):
        cols_sb, rows_sb, invdeg_sb, w0_sb, w1_sb, iota_sb = consts
        for sb in range(N_SB):
            tiles = [t for t in range(sb * SB_TILES, min((sb + 1) * SB_TILES, T_TILES))]
            g0 = (sb * n_banks) * SB_TILES
            g_end = ((sb + 1) * n_banks) * SB_TILES
            base = off[g0]
            k_sb = int(sum(K[g0:g_end]))
            if k_sb == 0:
                continue
            feat = fpool.tile([P, k_sb * (D_IN // 2)], F32, tag="feat")
            featv = feat[:].bitcast(F16)  # [P, k_sb * D_IN] fp16 view
            for b in range(n_banks):
                gb = (sb * n_banks + b) * SB_TILES
                kb = int(sum(K[gb : gb + SB_TILES]))
                if kb == 0:
                    continue
                rel = off[gb] - base
                dh = D_IN // 2
                nc.gpsimd.dma_gather(
                    out_ap=feat[:, rel * dh : (rel + kb) * dh].rearrange(
                        "p (c d) -> p c d", c=kb
                    ),
                    in_ap=h_d.ap()[
                        b * bank_rows : min((b + 1) * bank_rows, n_table), :
                    ],
                    idxs_ap=cols_sb[:, off[gb] * 8 : (off[gb] + kb) * 8],
                    num_idxs=kb * P,
                    num_idxs_reg=kb * P,
                    elem_size=dh,
                    single_packet=False,
                    queue_num=b,
                )
            sel = spool.tile([P, k_sb * P], F16, tag="sel")
            nc.vector.tensor_tensor(
                out=sel[:].rearrange("p (k n) -> p k n", k=k_sb),
                in0=iota_sb[:].unsqueeze(1).to_broadcast([P, k_sb, P]),
                in1=rows_sb[:, base : base + k_sb]
                .unsqueeze(2)
                .to_broadcast([P, k_sb, P]),
                op=mybir.AluOpType.is_equal,
            )
            # chunk -> tile-position map and first/last chunk per tpos
            chunk_tpos = []
            for b in range(n_banks):
                for tpos in range(len(tiles)):
                    g = (sb * n_banks + b) * SB_TILES + tpos
                    chunk_tpos += [tpos] * int(K[g])
            first = {}
            last = {}
            for j, tp in enumerate(chunk_tpos):
                first.setdefault(tp, j)
                last[tp] = j
            psums = {}
            for tp in sorted(first):
                p0 = ppool.tile([P, P], F32, tag="p0")
                p1 = ppool.tile([P, P], F32, tag="p1")
                psums[tp] = (p0, p1)
            for j, tp in enumerate(chunk_tpos):
                fj = featv[:, j * D_IN : (j + 1) * D_IN]
                sj = sel[:, j * P : (j + 1) * P]
                p0, p1 = psums[tp]
                nc.tensor.matmul(
                    out=p0[:],
                    lhsT=fj[:, :P],
                    rhs=sj,
                    start=(j == first[tp]),
                    stop=(j == last[tp]),
                )
                nc.tensor.matmul(
                    out=p1[:],
                    lhsT=fj[:, P:],
                    rhs=sj,
                    start=(j == first[tp]),
                    stop=(j == last[tp]),
                )
            for tp in sorted(first):
                t = tiles[tp]
                p0, p1 = psums[tp]
                a0 = apool.tile([P, P], F32, tag="a0")
                a1 = apool.tile([P, P], F32, tag="a1")
                nc.scalar.copy(out=a0[:], in_=p0[:])
                nc.scalar.copy(out=a1[:], in_=p1[:])
                po = ppool.tile([P, DCAT], F32, tag="po")
                nc.tensor.matmul(
                    out=po[:], lhsT=a0[:], rhs=w0_sb[:], start=True, stop=False
                )
                nc.tensor.matmul(
                    out=po[:], lhsT=a1[:], rhs=w1_sb[:], start=False, stop=True
                )
                ot = opool.tile([P, DCAT], F32, tag="ot")
                nc.scalar.activation(
                    out=ot[:],
                    in_=po[:],
                    func=mybir.ActivationFunctionType.Copy,
                    scale=invdeg_sb[:, t : t + 1],
                )
                nc.sync.dma_start(out=out_d.ap()[t * P : (t + 1) * P, :], in_=ot[:])

    with tile.TileContext(nc) as tc:
        with (
            tc.tile_pool(name="const", bufs=1) as cpool,
            tc.tile_pool(name="feat", bufs=3) as fpool,
            tc.tile_pool(name="sel", bufs=3) as spool,
            tc.tile_pool(name="agg", bufs=2) as apool,
            tc.tile_pool(name="outp", bufs=3) as opool,
            tc.tile_pool(name="psum", bufs=2, space="PSUM") as ppool,
        ):
            cols_sb = cpool.tile([P, chunks_total * 8], I16)
            rows_sb = cpool.tile([P, chunks_total], F16)
            invdeg_sb = cpool.tile([P, T_TILES], F32)
            w0_sb = cpool.tile([P, DCAT], F32, tag="w0")
            w1_sb = cpool.tile([P, DCAT], F32, tag="w1")
            iota_sb = cpool.tile([P, P], F16)
            nc.sync.dma_start(out=cols_sb[:], in_=cols_d.ap())
            nc.sync.dma_start(out=rows_sb[:], in_=rows_d.ap())
            nc.sync.dma_start(out=invdeg_sb[:], in_=invdeg_d.ap())
            nc.sync.dma_start(out=w0_sb[:], in_=wcat_d.ap()[0])
            nc.sync.dma_start(out=w1_sb[:], in_=wcat_d.ap()[1])
            nc.sync.dma_start(out=iota_sb[:], in_=iota_d.ap())
            consts = (cols_sb, rows_sb, invdeg_sb, w0_sb, w1_sb, iota_sb)
            if repeat == 1:
                body(tc, cpool, fpool, spool, apool, opool, ppool, consts)
            else:
                with tc.For_i(0, repeat, 1):
                    body(tc, cpool, fpool, spool, apool, opool, ppool, consts)
    nc.compile()
    return nc


def _prep_inputs(h, W, edge_idx, n_nodes, nodes_per_core, n_cores, bank_rows):
    h16 = np.ascontiguousarray(np.asarray(h, dtype=np.float32).astype(NP_FEAT))
    h_wire = h16.view(np.float32)  # fp16 payload shipped as f32[N, D_IN//2]
    W = np.asarray(W, dtype=np.float32)
    ei = np.asarray(edge_idx)
    row = ei[0].astype(np.int64)
    col = ei[1].astype(np.int64)
    e = row.shape[0]
    n_banks = N_BANKS

    deg = np.bincount(row, minlength=n_nodes).astype(np.float64)
    invdeg_full = np.where(deg > 0, 1.0 / np.maximum(deg, 1.0), 0.0).astype(np.float32)

    core = row // nodes_per_core
    local = row - core * nodes_per_core
    tloc = local // P
    slot = local - tloc * P
    sbk = tloc // SB_TILES
    tpos = tloc - sbk * SB_TILES
    bank = col // bank_rows
    colloc = (col - bank * bank_rows).astype(np.int16)

    n_groups = N_SB * n_banks * SB_TILES
    gkey_local = (sbk * n_banks + bank) * SB_TILES + tpos
    gkey = core * n_groups + gkey_local
    order = np.argsort(gkey, kind="stable")
    gkey_s = gkey[order]
    counts = np.bincount(gkey_s, minlength=n_cores * n_groups).reshape(
        n_cores, n_groups
    )
    K = np.ceil(counts.max(axis=0) / P).astype(np.int64)  # [n_groups]
    off = np.zeros(n_groups, dtype=np.int64)
    np.cumsum(K[:-1], out=off[1:])
    chunks_total = int(K.sum())

    starts = np.zeros(n_cores * n_groups + 1, dtype=np.int64)
    np.cumsum(counts.reshape(-1), out=starts[1:])
    pos = np.arange(e, dtype=np.int64) - starts[gkey_s]
    chunkpos = pos // P
    part = pos - chunkpos * P

    g_s = gkey_local[order]
    gchunk = off[g_s] + chunkpos

    # gather-call grouping: call (sb, b) covers groups [gb, gb+SB_TILES);
    # idx position within the call is relative to off[gb].
    gb_s = (g_s // SB_TILES) * SB_TILES
    i_call = (gchunk - off[gb_s]) * P + part

    cols_arr = np.zeros((n_cores, P, chunks_total * 8), dtype=np.int16)
    rows_arr = np.full((n_cores, P, chunks_total), 32768.0, dtype=NP_FEAT)
    core_s = core[order]
    idx_col = off[gb_s] * 8 + i_call // 16
    idx_row = i_call % 16
    colloc_s = colloc[order]
    for grp in range(8):  # replicate across the 8 q7 cores
        cols_arr[core_s, grp * 16 + idx_row, idx_col] = colloc_s
    rows_arr[core_s, part, gchunk] = slot[order].astype(NP_FEAT)

    invdeg_arr = np.zeros((n_cores, P, T_TILES), dtype=np.float32)
    nloc = np.arange(nodes_per_core, dtype=np.int64)
    tt = nloc // P
    ss = nloc - tt * P
    for c in range(n_cores):
        invdeg_arr[c, ss, tt] = invdeg_full[c * nodes_per_core + nloc]

    wcat = np.ascontiguousarray(
        W.transpose(1, 0, 2).reshape(D_IN, DCAT).reshape(2, P, DCAT)
    )
    iota_row = np.tile(np.arange(P, dtype=NP_FEAT)[None, :], (P, 1))

    in_maps = [
        {
            "h": h_wire,
            "cols": cols_arr[c],
            "rows": rows_arr[c],
            "invdeg": invdeg_arr[c],
            "wcat": wcat,
            "iota": iota_row,
        }
        for c in range(n_cores)
    ]
    return in_maps, K.tolist(), off.tolist()


_NC_CACHE: dict = {}


def kernel(h, W, edge_idx) -> np.ndarray:
    in_maps, K, off = _prep_inputs(
        h, W, edge_idx, N_NODES, NODES_PER_CORE, N_CORES, BANK
    )
    key = tuple(K)
    if key not in _NC_CACHE:
        _NC_CACHE[key] = _build_program(N_NODES, BANK, K, off, N_CORES)
    nc = _NC_CACHE[key]
    res = run_bass_kernel_spmd(nc, in_maps, core_ids=list(range(N_CORES)))
    out = np.concatenate(
        [res.results[c]["out"][:NODES_PER_CORE] for c in range(N_CORES)], axis=0
    )
    return np.ascontiguousarray(out.astype(np.float32))

